# revision 34
# baseline (speedup 1.0000x reference)
"""Trainium2 Bass kernel for CharModel ragged segment-mean + pos embedding.

Computation (per sample):
  out[j, :] = mean(feats[start_j:end_j, :]) * valid_j + pos_table[pos_j]
where the ragged segments are given by sorted word start offsets.

Strategy (bf16 data path, fp32 PSUM accumulate):
  - Host precomputes per-char metadata: word_id[c] (which word each char
    belongs to, -1 for padding chars) and wrec[c] = 1/len(word(c)).
  - Device builds a one-hot matrix M[c, j] = (word_id[c]==j) in bf16 with
    one fused DVE tensor_scalar op per 128-char tile, then the PE
    accumulates sum[W, D] = M.T @ feats_bf16 in PSUM; the pos embedding
    is added by accumulating (onehot(pos)*len).T @ pos_table into the same
    PSUM, and the PSUM->SBUF copy scales everything by 1/len.  bf16 feats
    bound the error at ~1e-3 relative, well inside the 2e-2 gate, and
    halve both the feats HBM traffic and the PE matmul count vs. the
    fp32-via-hi/lo-pair scheme.
  - Data parallel over batch: 8 NeuronCores x 4 samples each, one shared
    SPMD program, per-core input maps.

Walrus ISA wait-slot limits dealt with throughout: matmul (S3_LW),
tensor_scalar (S3D3_TS) and DMA (PSEUDO_DMA_DIRECT2D) instructions can
carry only ONE semaphore wait each.  Hence:
  - word metadata (cpk, 24KB) rides the SP HWDGE queue as its first
    trigger; the pos pack (cpb+table, one merged bf16 tensor) rides the
    ACT HWDGE queue; the iota row is generated on-device (Pool iota), so
    the tiny constants clear the DMA rings well before the feats bulk.
    HWDGE ring sems are global round-robin, so any late HWDGE DMA gets a
    ring-reuse wait on top of its data wait - outputs go via SWDGE only;
  - a per-sample 1x1 "gate" matmul (forced first in PE order via
    add_dep_helper, writing a dedicated never-overlapping PSUM region)
    alone carries the DVE wait for the lhsT builds and, transitively, the
    previous sample's PSUM bank releases;
  - the two D-chunks of a (group, tile) pair run back-to-back sharing one
    stationary load, which hides the ~226ns weight-load latency;
  - the PE warmup runs on a Pool-memset tile (dependency-free, so it
    trips the clock ramp during the init barriers' aftermath) and its
    PSUM lives in the matmul pool (frees a bank for a deeper rotation);
  - feats tiles and output staging tiles get enough pool bufs that no
    slot is ever reused (no WAR waits on DMAs/copies);
  - two output DMAs per sample (8 SWDGE DMAs over 8 queues -> no
    queue-FIFO reuse waits alongside the data wait), each gated by a Pool
    probe that folds the DVE copy wait into the Pool clock.
"""

import sys

if "/opt/trn_rl_repo" not in sys.path:
    sys.path.insert(0, "/opt/trn_rl_repo")

import numpy as np

import bass_rust
import concourse.bass as bass
import concourse.mybir as mybir
from concourse.tile import TileContext
from concourse.tile_sem_assignment import N_PROCS


class ChunkedDrainTileContext(TileContext):
    """TileContext whose kernel-tail drain is split into several drain
    instructions with one sem wait each (the CTRL_NO ISA struct rejects
    multi-wait drains here)."""

    DRAIN_CHUNK = 1

    def _drain_and_barrier(self, tick_clock, wait_clock):
        gc = tick_clock.global_clock
        ticks = [gc.peek_next(i) - 1 for i in range(N_PROCS)]
        active = [i for i, t in enumerate(ticks) if t > 0]
        for i in range(0, len(active), self.DRAIN_CHUNK):
            chunk = set(active[i : i + self.DRAIN_CHUNK])
            part = [ticks[j] if j in chunk else 0 for j in range(N_PROCS)]
            d = self.nc.sync.drain()
            wait_clock.add_sem_waits(
                d.ins, bass_rust.ScopedClock({None: bass_rust.VectorClock(part)})
            )
        self.nc.all_engine_barrier()
        assert self.sems is not None
        popped = self.nc._tile_sem_poison_stack.pop()
        assert popped is self._sem_poison
        self.nc.clear_and_free_semaphores(list(self.sems.allocated().values()))
        self.nc.all_engine_barrier()

B, S, D, W, NPOS = 32, 1024, 512 + 256, 512, 32  # D=768
N_CORES = 8
SPC = B // N_CORES  # samples per core
NT = S // 128  # char tiles per sample
NG = W // 128  # word groups per sample
CHUNKS = ((0, 384), (384, 384))  # D split for PSUM bank limit
F32 = mybir.dt.float32

BF16 = mybir.dt.bfloat16

# constpack layout ([128, CPK_W] f32; iota is generated on-device so this
# stays tiny and clears the DMA rings ahead of the feats bulk)
CPK_META = 0  # [128, NT*SPC]: per sample s: word-id per char, tile cols
CPK_WREC = NT * SPC  # [128, NG*SPC]: per sample s: 1/len per word
CPK_W = CPK_WREC + NG * SPC

# bf16 pos pack: partitions 0:32 (= NPOS); per-sample one-hot column blocks
# followed by one shared pos_table copy (single DMA -> ACT queue stays at 3
# DMAs total, inside its 3 ring slots, so no queue-reuse waits)
CPB_TAB = SPC * W
CPB_W = SPC * W + D


def _build_program(sched):
    """sched[s][g] = tuple of char-tile indices whose chars can touch word
    group g of slot-s samples on ANY core (union schedule; the one-hot
    lhsT zeroes contributions from tiles/words not actually present on a
    given core).  Matmuls for (g, t) pairs outside the schedule multiply
    all-zero one-hot slices and are skipped entirely."""
    nc = bass.Bass()
    feats = nc.declare_dram_parameter("feats", [SPC, S, D], BF16, False)
    constpack = nc.declare_dram_parameter("constpack", [128, CPK_W], F32, False)
    constpkb = nc.declare_dram_parameter("constpkb", [32, CPB_W], BF16, False)
    out = nc.declare_dram_parameter("out", [SPC, W, D], BF16, True)

    dep = lambda a, b, why: bass_rust.add_dep_helper(
        a.ins, b.ins, sync=False, reason=why
    )

    n_lh = sum(
        len({t for g in range(NG) for t in sched[s][g]}) for s in range(SPC)
    )
    # Coalesce each sample's used char tiles into contiguous runs (max 4
    # tiles) -> one 3D-AP DMA per run: ~8 big DMAs instead of ~32, one per
    # HWDGE queue, so the SP sequencer's ~1us per-DMA issue cost stops
    # dominating the kernel head.
    MAXRUN = 4
    all_runs = {}
    from collections import Counter

    runcnt = Counter()
    for s in range(SPC):
        uts = sorted({t for g in range(NG) for t in sched[s][g]})
        runs = []
        i = 0
        while i < len(uts):
            j = i
            while (
                j + 1 < len(uts)
                and uts[j + 1] == uts[j] + 1
                and (j + 1 - i) < MAXRUN
            ):
                j += 1
            runs.append((uts[i], j - i + 1))
            i = j + 1
        all_runs[s] = runs
        for (_, L) in runs:
            runcnt[L] += 1
    with ChunkedDrainTileContext(nc) as tc:
        with (
            tc.tile_pool(name="const", bufs=1) as cpool,
            tc.tile_pool(name="feat", bufs=SPC * NT) as fpool,
            tc.tile_pool(name="lhs", bufs=n_lh) as lpool,
            tc.tile_pool(name="outsb", bufs=SPC) as opool,
            tc.tile_pool(name="psum", bufs=2 * NG - 1, space="PSUM") as ppool,
            tc.tile_pool(name="gatep", bufs=1, space="PSUM") as gpool,
        ):
            # Constants go over the Activation engine's HWDGE queue: it is
            # otherwise idle at kernel start, triggers immediately after the
            # init barriers, and does not share a completion semaphore with
            # the bulk feats traffic (the SWDGE path ticked ~6us after the
            # data actually landed, stalling every dependent).
            # cpk rides the SP queue as its very first trigger (24KB, done
            # before the feats bulk); cpb rides the ACT queue, whose 2nd
            # ring slot is saved for the tail-critical final output DMA.
            cpk = cpool.tile([128, CPK_W], F32)
            nc.sync.dma_start(out=cpk[:, :], in_=constpack[:, :])
            cpb = cpool.tile([32, CPB_W], BF16)
            nc.scalar.dma_start(out=cpb[:, :], in_=constpkb[:, :])
            tab = cpb[:, CPB_TAB : CPB_TAB + D]
            # iota row 0..W-1 generated on the Pool engine (f32 is exact for
            # 0..511) while the constpack DMAs are in flight.  A DVE probe
            # observes the Pool tick so the lhsT builds carry only the
            # constpack-DMA wait (one sem slot per tensor_scalar).
            iota_f_t = cpool.tile([128, W], F32)
            nc.gpsimd.iota(
                iota_f_t[:, :],
                [[1, W]],
                channel_multiplier=0,
                allow_small_or_imprecise_dtypes=True,
            )
            iota_f = iota_f_t[:, :]
            dve_probe = cpool.tile([1, 1], F32)
            nc.vector.tensor_copy(dve_probe[0:1, 0:1], iota_f_t[0:1, 0:1])
            # ACT probe: observe the constpack DMA tick on the Scalar engine
            # so the per-unit ACT output copies carry only their PE wait.
            act_probe = cpool.tile([1, 1], F32)
            nc.scalar.copy(act_probe[0:1, 0:1], cpk[0:1, 0:1])
            pl_probe = cpool.tile([1, 2 * SPC], BF16)
            ap_probe = cpool.tile([1, 1], BF16)
            # PE warm-up: dependency-free bf16 matmuls on a Pool-memset tile
            # start right after the init barriers and trip the HAM clock gate
            # to K=8/8 before the real matmuls start.  Without this the PE
            # sometimes stays at 1.2GHz for the whole kernel.
            wconst = cpool.tile([128, 512], BF16)
            nc.gpsimd.memset(wconst[:, :], 1.0)
            # Warmup PSUM shares the matmul pool (frees one PSUM bank for a
            # deeper rotation; its buf is recycled by a late unit of s0).
            wps = ppool.tile([128, 384], F32, tag="ps", name="warm")
            for wi in range(10):
                nc.tensor.matmul(
                    wps[:, :],
                    wconst[:, 0:128],
                    wconst[:, 0:384],
                    start=(wi == 0),
                    stop=(wi == 9),
                    skip_group_check=True,
                )
            # One persistent PSUM bank for the gates; each gate writes a
            # disjoint region so gates never carry a WAW drain wait.
            gate_t = gpool.tile([128, 4 * SPC], F32)

            prev_ob = None  # previous sample's output staging buffer
            for s in range(SPC):
                last_dve_copy = None
                last_act_copy = None
                used_tiles = sorted({t for g in range(NG) for t in sched[s][g]})
                fts, lhs = {}, {}
                first_build = True
                for (t0, L) in all_runs[s]:
                    ftr = fpool.tile(
                        [128, L, D],
                        BF16,
                        tag=f"ftr{L}",
                        bufs=runcnt[L],
                        name=f"ftr_{s}_{t0}",
                    )
                    nc.sync.dma_start(
                        out=ftr[:, :, :],
                        in_=feats[s, 128 * t0 : 128 * (t0 + L), :].rearrange(
                            "(i p) d -> p i d", p=128
                        ),
                    )
                    for i in range(L):
                        fts[t0 + i] = ftr[:, i, :]
                lhs_gl = {}
                for t in used_tiles:
                    # Only build the one-hot columns of the word groups this
                    # tile actually feeds (the matmuls read nothing else);
                    # halves the DVE build cost.
                    gs = [g for g in range(NG) if t in sched[s][g]]
                    gl, gh = min(gs), max(gs)
                    lhs_gl[t] = gl
                    lh = lpool.tile([128, W], BF16, tag="lh", name=f"lh_{s}_{t}")
                    wcol = CPK_META + NT * s
                    nc.vector.tensor_scalar(
                        lh[:, 128 * gl : 128 * (gh + 1)],
                        iota_f[:, 128 * gl : 128 * (gh + 1)],
                        cpk[:, wcol + t : wcol + t + 1],
                        None,
                        op0=mybir.AluOpType.is_equal,
                    )
                    lhs[t] = lh

                # Gate A: 1x1x1 matmul reading the last lhsT build; forced
                # first in PE order so it alone carries the DVE wait for
                # this sample's builds.
                t_last = used_tiles[-1]
                last_lh = lhs[t_last]
                lc = 128 * lhs_gl[t_last]
                gate = nc.tensor.matmul(
                    gate_t[0:1, s : s + 1],
                    last_lh[0:1, lc : lc + 1],
                    last_lh[0:1, lc : lc + 1],
                    start=True,
                    stop=True,
                    skip_group_check=True,
                )
                gates = [gate]
                # Gate B: reads the previous sample's output staging buffer
                # (written by its LAST PSUM->SBUF op, which a dep chain keeps
                # last on DVE), so this one wait covers all of the previous
                # sample's PSUM bank releases.  The builds above no longer
                # depend on those releases, so they run early and the PE does
                # not stall at sample boundaries.
                if prev_ob is not None:
                    # last DVE copy writes unit 6 -> ob[:, 3D : 3D+384];
                    # last ACT copy writes unit 7 -> ob[:, 3D+384 : 4D]
                    bgate = nc.tensor.matmul(
                        gate_t[0:1, SPC + s : SPC + s + 1],
                        prev_ob[0:1, 3 * D : 3 * D + 1],
                        prev_ob[0:1, 3 * D : 3 * D + 1],
                        start=True,
                        stop=True,
                        skip_group_check=True,
                    )
                    bgate2 = nc.tensor.matmul(
                        gate_t[0:1, 2 * SPC + s : 2 * SPC + s + 1],
                        prev_ob[0:1, NG * D - 1 : NG * D],
                        prev_ob[0:1, NG * D - 1 : NG * D],
                        start=True,
                        stop=True,
                        skip_group_check=True,
                    )
                else:
                    bgate = bgate2 = None

                ob = opool.tile([128, NG * D], BF16, tag="ob", name=f"ob_{s}")
                for g in range(NG):
                    tiles_g = sched[s][g]
                    # Both D-chunks of a (group, tile) pair run back-to-back
                    # sharing one stationary load (the one-hot slice), so the
                    # ~226ns weight load hides under 2x~160ns of streaming.
                    pss = [
                        ppool.tile([128, cn], F32, tag="ps", name=f"ps_{s}_{g}_{ci}")
                        for ci, (c0, cn) in enumerate(CHUNKS)
                    ]
                    for k, t in enumerate(tiles_g):
                        for ci, (c0, cn) in enumerate(CHUNKS):
                            mm = nc.tensor.matmul(
                                pss[ci][:, :],
                                lhs[t][:, 128 * g : 128 * (g + 1)],
                                fts[t][:, c0 : c0 + cn],
                                start=(k == 0),
                                stop=False,
                                skip_group_check=True,
                            )
                            for gx in gates:
                                dep(mm, gx, "matmuls after sample gate")
                            if bgate is not None:
                                dep(mm, bgate, "matmuls after bank gate")
                                dep(mm, bgate2, "matmuls after bank gate2")
                    # pos contribution scaled by len so the final 1/len
                    # multiply leaves exactly pos_table[pos].
                    pc = W * s + 128 * g
                    for ci, (c0, cn) in enumerate(CHUNKS):
                        mm = nc.tensor.matmul(
                            pss[ci][:, :],
                            cpb[0:32, pc : pc + 128],
                            tab[:, c0 : c0 + cn],
                            start=(len(tiles_g) == 0),
                            stop=True,
                            skip_group_check=True,
                        )
                        for gx in gates:
                            dep(mm, gx, "pos matmul after sample gate")
                        if bgate is not None:
                            dep(mm, bgate, "pos matmul after bank gate")
                            dep(mm, bgate2, "pos matmul after bank gate2")
                    for ci, (c0, cn) in enumerate(CHUNKS):
                        unit = 2 * g + ci
                        recip_ap = cpk[
                            :, CPK_WREC + NG * s + g : CPK_WREC + NG * s + g + 1
                        ]
                        if unit % 2 == 0:
                            cp = nc.vector.tensor_scalar(
                                ob[:, g * D + c0 : g * D + c0 + cn],
                                pss[ci][:, :],
                                recip_ap,
                                None,
                                op0=mybir.AluOpType.mult,
                            )
                            if last_dve_copy is not None:
                                dep(cp, last_dve_copy, "DVE copy order")
                            last_dve_copy = cp
                        else:
                            cp = nc.scalar.activation(
                                ob[:, g * D + c0 : g * D + c0 + cn],
                                pss[ci][:, :],
                                mybir.ActivationFunctionType.Copy,
                                scale=recip_ap,
                            )
                            if last_act_copy is not None:
                                dep(cp, last_act_copy, "ACT copy order")
                            last_act_copy = cp
                # Pool probes observe the last DVE copy of each output half
                # on the Pool engine so each output DMA carries only its ACT
                # copy wait.  Two DMAs per sample pipeline the writeback:
                # groups 0-1 ship while groups 2-3 are still being copied.
                nc.gpsimd.tensor_copy(
                    pl_probe[0:1, 2 * s : 2 * s + 1], ob[0:1, D : D + 1]
                )
                nc.gpsimd.dma_start(
                    out=out[s, 0 : 2 * 128].rearrange("(g p) d -> p g d", p=128),
                    in_=ob[:, 0 : 2 * D].rearrange("p (g d) -> p g d", g=2),
                )
                nc.gpsimd.tensor_copy(
                    pl_probe[0:1, 2 * s + 1 : 2 * s + 2],
                    ob[0:1, 3 * D : 3 * D + 1],
                )
                nc.gpsimd.dma_start(
                    out=out[s, 2 * 128 : 4 * 128].rearrange(
                        "(g p) d -> p g d", p=128
                    ),
                    in_=ob[:, 2 * D : 4 * D].rearrange("p (g d) -> p g d", g=2),
                )
                prev_ob = ob
    return nc


_PROGRAM_CACHE = {}


def _get_program(sched):
    key = tuple(tuple(tuple(g) for g in s) for s in sched)
    if key not in _PROGRAM_CACHE:
        _PROGRAM_CACHE[key] = _build_program(sched)
    return _PROGRAM_CACHE[key]


def _assign_slots(spans):
    """Assign the B samples to (slot, core) so that the per-slot UNION of
    (group, char-tile) matmul footprints is small: sort by profile, then
    local-search swaps with incremental per-slot cost."""
    import random

    masks = []
    for i in range(B):
        u = np.zeros((NG, NT), bool)
        for (g, t0, t1) in spans[i][0]:
            u[g, t0 : t1 + 1] = True
        masks.append(u)

    def slot_cost(slot):
        u = np.zeros((NG, NT), bool)
        for i in slot:
            u |= masks[i]
        return int(u.sum())

    order = sorted(range(B), key=lambda i: spans[i][1])
    assign = [[order[s * N_CORES + c] for c in range(N_CORES)] for s in range(SPC)]
    costs = [slot_cost(slot) for slot in assign]
    rng = random.Random(0)
    for _ in range(60000):
        s1, s2 = rng.randrange(SPC), rng.randrange(SPC)
        if s1 == s2:
            continue
        i1, i2 = rng.randrange(N_CORES), rng.randrange(N_CORES)
        assign[s1][i1], assign[s2][i2] = assign[s2][i2], assign[s1][i1]
        c1, c2 = slot_cost(assign[s1]), slot_cost(assign[s2])
        if c1 + c2 <= costs[s1] + costs[s2]:
            costs[s1], costs[s2] = c1, c2
        else:
            assign[s1][i1], assign[s2][i2] = assign[s2][i2], assign[s1][i1]
    return assign


def _prep_inputs(feats, word_lens, seq_len, pos, pos_table):
    """Host-side metadata prep + batch sharding -> per-core input maps,
    union matmul schedule, and the sample->(slot, core) assignment."""
    feats = np.ascontiguousarray(np.asarray(feats), dtype=np.float32)
    word_lens = np.asarray(word_lens).astype(np.int64)
    seq_len = np.asarray(seq_len).astype(np.int64)
    pos = np.asarray(pos).astype(np.int64)
    pos_table = np.ascontiguousarray(np.asarray(pos_table), dtype=np.float32)

    import ml_dtypes

    bf16 = ml_dtypes.bfloat16
    wid = np.full((B, S), -1.0, np.float32)
    wrecw = np.zeros((B, W), np.float32)  # 1/len per word (0 for padding)
    lenw = np.zeros((B, W), np.float32)  # len per word (0 for padding)
    posoh = np.zeros((B, NPOS, W), np.float32)
    spans = []  # per sample: ([(g, t0, t1), ...], profile_key)
    for i in range(B):
        wl = word_lens[i]
        sl = int(seq_len[i])
        valid = wl != 0
        valid[0] = True
        ridx = np.nonzero(valid)[0]  # real words (contiguous prefix by construction)
        starts = wl[ridx]
        n = len(ridx)
        nxt = np.append(starts[1:], 0)
        ends = np.where(nxt == 0, sl, nxt)
        lens = np.maximum(ends - starts, 1)
        cidx = np.arange(sl)
        cwid = np.searchsorted(starts, cidx, side="right") - 1
        wid[i, :sl] = ridx[cwid].astype(np.float32)
        wrecw[i, ridx] = 1.0 / lens.astype(np.float32)
        lenw[i, ridx] = lens.astype(np.float32)
        posoh[i, pos[i], np.arange(W)] = 1.0  # one-hot part
        sp = []
        for g in range(NG):
            w0 = 128 * g
            if w0 >= n:
                continue
            w1 = min(128 * (g + 1), n)
            c0, c1 = int(starts[w0]), int(ends[w1 - 1])
            sp.append((g, c0 // 128, (c1 - 1) // 128))
        spans.append((sp, (n, sl)))

    assign = _assign_slots(spans)
    sched = []
    for s in range(SPC):
        u = np.zeros((NG, NT), bool)
        for i in assign[s]:
            for (g, t0, t1) in spans[i][0]:
                u[g, t0 : t1 + 1] = True
        sched.append(tuple(tuple(np.nonzero(u[g])[0].tolist()) for g in range(NG)))
    sched = tuple(sched)

    # [B, S] -> [B, 128, NT]: per-partition scalar columns per char tile
    widT = wid.reshape(B, NT, 128).transpose(0, 2, 1)
    # 1/len per word -> [B, 128, NG] per-partition scalars per word group
    wrecwT = wrecw.reshape(B, NG, 128).transpose(0, 2, 1)

    feats_bf = feats.astype(bf16)
    tab_bf = np.ascontiguousarray(pos_table.astype(bf16))

    in_maps = []
    for c in range(N_CORES):
        cpk = np.zeros((128, CPK_W), np.float32)
        cpb = np.zeros((32, CPB_W), bf16)
        cpb[:, CPB_TAB : CPB_TAB + D] = tab_bf
        feats_c = np.empty((SPC, S, D), bf16)
        for s in range(SPC):
            i = assign[s][c]
            feats_c[s] = feats_bf[i]
            cpk[:, CPK_META + NT * s : CPK_META + NT * (s + 1)] = widT[i]
            cpk[:, CPK_WREC + NG * s : CPK_WREC + NG * (s + 1)] = wrecwT[i]
            cpb[:, W * s : W * (s + 1)] = (posoh[i] * lenw[i][None, :]).astype(bf16)
        in_maps.append({"feats": feats_c, "constpack": cpk, "constpkb": cpb})
    return in_maps, sched, assign


def _run(in_maps, sched, assign, trace=False):
    from concourse.bass_utils import run_bass_kernel_spmd

    nc = _get_program(sched)
    res = run_bass_kernel_spmd(nc, in_maps, list(range(N_CORES)), trace=trace)
    out = np.empty((B, W, D), np.float32)
    for c in range(N_CORES):
        for s in range(SPC):
            out[assign[s][c]] = res.results[c]["out"][s].astype(np.float32)
    return out, res


def kernel(feats, word_lens, seq_len, pos, pos_table):
    in_maps, sched, assign = _prep_inputs(feats, word_lens, seq_len, pos, pos_table)
    out, _ = _run(in_maps, sched, assign, trace=False)
    return out



# revision 37
# speedup vs baseline: 1.0734x; 1.0734x over previous
"""Trainium2 Bass kernel for CharModel ragged segment-mean + pos embedding.

Computation (per sample):
  out[j, :] = mean(feats[start_j:end_j, :]) * valid_j + pos_table[pos_j]
where the ragged segments are given by sorted word start offsets.

Strategy (bf16 data path, fp32 PSUM accumulate):
  - Host precomputes per-char metadata: word_id[c] (which word each char
    belongs to, -1 for padding chars) and wrec[c] = 1/len(word(c)).
  - Device builds a one-hot matrix M[c, j] = (word_id[c]==j) in bf16 with
    one fused DVE tensor_scalar op per 128-char tile, then the PE
    accumulates sum[W, D] = M.T @ feats_bf16 in PSUM; the pos embedding
    is added by accumulating (onehot(pos)*len).T @ pos_table into the same
    PSUM, and the PSUM->SBUF copy scales everything by 1/len.  bf16 feats
    bound the error at ~1e-3 relative, well inside the 2e-2 gate, and
    halve both the feats HBM traffic and the PE matmul count vs. the
    fp32-via-hi/lo-pair scheme.
  - Data parallel over batch: 8 NeuronCores x 4 samples each, one shared
    SPMD program, per-core input maps.

Walrus ISA wait-slot limits dealt with throughout: matmul (S3_LW),
tensor_scalar (S3D3_TS) and DMA (PSEUDO_DMA_DIRECT2D) instructions can
carry only ONE semaphore wait each.  Hence:
  - word metadata (cpk, 24KB) rides the SP HWDGE queue as its first
    trigger; the pos pack (cpb+table, one merged bf16 tensor) rides the
    ACT HWDGE queue; the iota row is generated on-device (Pool iota), so
    the tiny constants clear the DMA rings well before the feats bulk.
    HWDGE ring sems are global round-robin, so any late HWDGE DMA gets a
    ring-reuse wait on top of its data wait - outputs go via SWDGE only;
  - a per-sample 1x1 "gate" matmul (forced first in PE order via
    add_dep_helper, writing a dedicated never-overlapping PSUM region)
    alone carries the DVE wait for the lhsT builds and, transitively, the
    previous sample's PSUM bank releases;
  - the two D-chunks of a (group, tile) pair run back-to-back sharing one
    stationary load, which hides the ~226ns weight-load latency;
  - the PE warmup runs on a Pool-memset tile (dependency-free, so it
    trips the clock ramp during the init barriers' aftermath) and its
    PSUM lives in the matmul pool (frees a bank for a deeper rotation);
  - feats tiles and output staging tiles get enough pool bufs that no
    slot is ever reused (no WAR waits on DMAs/copies);
  - two output DMAs per sample (8 SWDGE DMAs over 8 queues -> no
    queue-FIFO reuse waits alongside the data wait), each gated by a Pool
    probe that folds the DVE copy wait into the Pool clock.
"""

import sys

if "/opt/trn_rl_repo" not in sys.path:
    sys.path.insert(0, "/opt/trn_rl_repo")

import numpy as np

import bass_rust
import concourse.bass as bass
import concourse.mybir as mybir
from concourse.tile import TileContext
from concourse.tile_sem_assignment import N_PROCS


class ChunkedDrainTileContext(TileContext):
    """TileContext whose kernel-tail drain is split into several drain
    instructions with one sem wait each (the CTRL_NO ISA struct rejects
    multi-wait drains here)."""

    DRAIN_CHUNK = 1

    def _drain_and_barrier(self, tick_clock, wait_clock):
        gc = tick_clock.global_clock
        ticks = [gc.peek_next(i) - 1 for i in range(N_PROCS)]
        active = [i for i, t in enumerate(ticks) if t > 0]
        for i in range(0, len(active), self.DRAIN_CHUNK):
            chunk = set(active[i : i + self.DRAIN_CHUNK])
            part = [ticks[j] if j in chunk else 0 for j in range(N_PROCS)]
            d = self.nc.sync.drain()
            wait_clock.add_sem_waits(
                d.ins, bass_rust.ScopedClock({None: bass_rust.VectorClock(part)})
            )
        self.nc.all_engine_barrier()
        assert self.sems is not None
        popped = self.nc._tile_sem_poison_stack.pop()
        assert popped is self._sem_poison
        self.nc.clear_and_free_semaphores(list(self.sems.allocated().values()))
        self.nc.all_engine_barrier()

B, S, D, W, NPOS = 32, 1024, 512 + 256, 512, 32  # D=768
N_CORES = 8
SPC = B // N_CORES  # samples per core
NT = S // 128  # char tiles per sample
NG = W // 128  # word groups per sample
CHUNKS = ((0, 384), (384, 384))  # D split for PSUM bank limit
F32 = mybir.dt.float32

BF16 = mybir.dt.bfloat16

# constpack layout ([128, CPK_W] f32; iota is generated on-device so this
# stays tiny and clears the DMA rings ahead of the feats bulk)
CPK_META = 0  # [128, NT*SPC]: per sample s: word-id per char, tile cols
CPK_WREC = NT * SPC  # [128, NG*SPC]: per sample s: 1/len per word
CPK_W = CPK_WREC + NG * SPC

# bf16 pos pack: partitions 0:32 (= NPOS); per-sample one-hot column blocks
# followed by one shared pos_table copy (single DMA -> ACT queue stays at 3
# DMAs total, inside its 3 ring slots, so no queue-reuse waits)
CPB_TAB = SPC * W
CPB_W = SPC * W + D


def _build_program(sched):
    """sched[s][g] = tuple of char-tile indices whose chars can touch word
    group g of slot-s samples on ANY core (union schedule; the one-hot
    lhsT zeroes contributions from tiles/words not actually present on a
    given core).  Matmuls for (g, t) pairs outside the schedule multiply
    all-zero one-hot slices and are skipped entirely."""
    nc = bass.Bass()
    feats = nc.declare_dram_parameter("feats", [SPC, S, D], BF16, False)
    constpack = nc.declare_dram_parameter("constpack", [128, CPK_W], F32, False)
    constpkb = nc.declare_dram_parameter("constpkb", [32, CPB_W], BF16, False)
    out = nc.declare_dram_parameter("out", [SPC, W, D], BF16, True)

    dep = lambda a, b, why: bass_rust.add_dep_helper(
        a.ins, b.ins, sync=False, reason=why
    )

    n_lh = sum(
        len({t for g in range(NG) for t in sched[s][g]}) for s in range(SPC)
    )
    # Coalesce each sample's used char tiles into contiguous runs (max 4
    # tiles) -> one 3D-AP DMA per run: ~8 big DMAs instead of ~32, one per
    # HWDGE queue, so the SP sequencer's ~1us per-DMA issue cost stops
    # dominating the kernel head.
    MAXRUN = 4
    all_runs = {}
    from collections import Counter

    runcnt = Counter()
    for s in range(SPC):
        uts = sorted({t for g in range(NG) for t in sched[s][g]})
        runs = []
        i = 0
        while i < len(uts):
            j = i
            while (
                j + 1 < len(uts)
                and uts[j + 1] == uts[j] + 1
                and (j + 1 - i) < MAXRUN
            ):
                j += 1
            runs.append((uts[i], j - i + 1))
            i = j + 1
        all_runs[s] = runs
        for (_, L) in runs:
            runcnt[L] += 1
    with ChunkedDrainTileContext(nc) as tc:
        with (
            tc.tile_pool(name="const", bufs=1) as cpool,
            tc.tile_pool(name="feat", bufs=SPC * NT) as fpool,
            tc.tile_pool(name="lhs", bufs=n_lh) as lpool,
            tc.tile_pool(name="outsb", bufs=SPC) as opool,
            tc.tile_pool(name="psum", bufs=2 * NG - 1, space="PSUM") as ppool,
            tc.tile_pool(name="gatep", bufs=1, space="PSUM") as gpool,
        ):
            # Constants go over the Activation engine's HWDGE queue: it is
            # otherwise idle at kernel start, triggers immediately after the
            # init barriers, and does not share a completion semaphore with
            # the bulk feats traffic (the SWDGE path ticked ~6us after the
            # data actually landed, stalling every dependent).
            # cpk rides the SP queue as its very first trigger (24KB, done
            # before the feats bulk); cpb rides the ACT queue, whose 2nd
            # ring slot is saved for the tail-critical final output DMA.
            cpk = cpool.tile([128, CPK_W], F32)
            nc.sync.dma_start(out=cpk[:, :], in_=constpack[:, :])
            cpb = cpool.tile([32, CPB_W], BF16)
            nc.scalar.dma_start(out=cpb[:, :], in_=constpkb[:, :])
            tab = cpb[:, CPB_TAB : CPB_TAB + D]
            # iota row 0..W-1 generated on the Pool engine (f32 is exact for
            # 0..511) while the constpack DMAs are in flight.  A DVE probe
            # observes the Pool tick so the lhsT builds carry only the
            # constpack-DMA wait (one sem slot per tensor_scalar).
            iota_f_t = cpool.tile([128, W], F32)
            nc.gpsimd.iota(
                iota_f_t[:, :],
                [[1, W]],
                channel_multiplier=0,
                allow_small_or_imprecise_dtypes=True,
            )
            iota_f = iota_f_t[:, :]
            dve_probe = cpool.tile([1, 1], F32)
            nc.vector.tensor_copy(dve_probe[0:1, 0:1], iota_f_t[0:1, 0:1])
            # ACT probe: observe the constpack DMA tick on the Scalar engine
            # so the per-unit ACT output copies carry only their PE wait.
            act_probe = cpool.tile([1, 1], F32)
            nc.scalar.copy(act_probe[0:1, 0:1], cpk[0:1, 0:1])
            pl_probe = cpool.tile([1, 2 * SPC], BF16)
            ap_probe = cpool.tile([1, 1], BF16)
            # PE warm-up: dependency-free bf16 matmuls on a Pool-memset tile
            # start right after the init barriers and trip the HAM clock gate
            # to K=8/8 before the real matmuls start.  Without this the PE
            # sometimes stays at 1.2GHz for the whole kernel.
            wconst = cpool.tile([128, 512], BF16)
            nc.gpsimd.memset(wconst[:, :], 1.0)
            # Warmup PSUM shares the matmul pool (frees one PSUM bank for a
            # deeper rotation; its buf is recycled by a late unit of s0).
            wps = ppool.tile([128, 384], F32, tag="ps", name="warm")
            for wi in range(12):
                nc.tensor.matmul(
                    wps[:, :],
                    wconst[:, 0:128],
                    wconst[:, 0:384],
                    start=(wi == 0),
                    stop=(wi == 11),
                    skip_group_check=True,
                )
            # One persistent PSUM bank for the gates; each gate writes a
            # disjoint region so gates never carry a WAW drain wait.
            gate_t = gpool.tile([128, 6 * SPC], F32)

            prev_ob = None  # previous sample's output staging buffer
            for s in range(SPC):
                last_dve_copy = None
                last_act_copy = None
                used_tiles = sorted({t for g in range(NG) for t in sched[s][g]})
                fts, lhs = {}, {}
                first_build = True
                for (t0, L) in all_runs[s]:
                    ftr = fpool.tile(
                        [128, L, D],
                        BF16,
                        tag=f"ftr{L}",
                        bufs=runcnt[L],
                        name=f"ftr_{s}_{t0}",
                    )
                    nc.sync.dma_start(
                        out=ftr[:, :, :],
                        in_=feats[s, 128 * t0 : 128 * (t0 + L), :].rearrange(
                            "(i p) d -> p i d", p=128
                        ),
                    )
                    for i in range(L):
                        fts[t0 + i] = ftr[:, i, :]
                lhs_gl = {}
                for t in used_tiles:
                    # Only build the one-hot columns of the word groups this
                    # tile actually feeds (the matmuls read nothing else);
                    # halves the DVE build cost.
                    gs = [g for g in range(NG) if t in sched[s][g]]
                    gl, gh = min(gs), max(gs)
                    lhs_gl[t] = gl
                    lh = lpool.tile([128, W], BF16, tag="lh", name=f"lh_{s}_{t}")
                    wcol = CPK_META + NT * s
                    nc.vector.tensor_scalar(
                        lh[:, 128 * gl : 128 * (gh + 1)],
                        iota_f[:, 128 * gl : 128 * (gh + 1)],
                        cpk[:, wcol + t : wcol + t + 1],
                        None,
                        op0=mybir.AluOpType.is_equal,
                    )
                    lhs[t] = lh

                # Gate A: 1x1x1 matmul reading the last lhsT build; forced
                # first in PE order so it alone carries the DVE wait for
                # this sample's builds.
                t_last = used_tiles[-1]
                last_lh = lhs[t_last]
                lc = 128 * lhs_gl[t_last]
                gate = nc.tensor.matmul(
                    gate_t[0:1, s : s + 1],
                    last_lh[0:1, lc : lc + 1],
                    last_lh[0:1, lc : lc + 1],
                    start=True,
                    stop=True,
                    skip_group_check=True,
                )
                gates = [gate]
                # Bank gates: with the 7-buf PSUM rotation, this sample's
                # unit k reuses the bank of the previous sample's unit k+1.
                # Units 0-3 therefore only need the prev sample's units 1-4
                # released: an EARLY gate pair witnesses the unit-4 DVE copy
                # (covers DVE units 0,2,4 via the copy dep chain) and the
                # unit-3 ACT copy (covers ACT units 1,3), both of which
                # complete well before the prev sample's last matmul -- so
                # this sample's groups 0-1 start with no boundary bubble.
                # A LATE pair (unit-6 DVE / unit-7 ACT witnesses) gates only
                # groups 2-3.
                if prev_ob is not None:
                    bgE = nc.tensor.matmul(
                        gate_t[0:1, SPC + s : SPC + s + 1],
                        prev_ob[0:1, 2 * D : 2 * D + 1],
                        prev_ob[0:1, 2 * D : 2 * D + 1],
                        start=True,
                        stop=True,
                        skip_group_check=True,
                    )
                    bgE2 = nc.tensor.matmul(
                        gate_t[0:1, 2 * SPC + s : 2 * SPC + s + 1],
                        prev_ob[0:1, 2 * D - 1 : 2 * D],
                        prev_ob[0:1, 2 * D - 1 : 2 * D],
                        start=True,
                        stop=True,
                        skip_group_check=True,
                    )
                else:
                    bgE = bgE2 = None
                bgL = bgL2 = None

                ob = opool.tile([128, NG * D], BF16, tag="ob", name=f"ob_{s}")
                for g in range(NG):
                    if g == 2 and prev_ob is not None:
                        bgL = nc.tensor.matmul(
                            gate_t[0:1, 4 * SPC + s : 4 * SPC + s + 1],
                            prev_ob[0:1, 3 * D : 3 * D + 1],
                            prev_ob[0:1, 3 * D : 3 * D + 1],
                            start=True,
                            stop=True,
                            skip_group_check=True,
                        )
                        bgL2 = nc.tensor.matmul(
                            gate_t[0:1, 5 * SPC + s : 5 * SPC + s + 1],
                            prev_ob[0:1, NG * D - 1 : NG * D],
                            prev_ob[0:1, NG * D - 1 : NG * D],
                            start=True,
                            stop=True,
                            skip_group_check=True,
                        )
                    bgate = bgE if g < 2 else bgL
                    bgate2 = bgE2 if g < 2 else bgL2
                    tiles_g = sched[s][g]
                    # Both D-chunks of a (group, tile) pair run back-to-back
                    # sharing one stationary load (the one-hot slice), so the
                    # ~226ns weight load hides under 2x~160ns of streaming.
                    pss = [
                        ppool.tile([128, cn], F32, tag="ps", name=f"ps_{s}_{g}_{ci}")
                        for ci, (c0, cn) in enumerate(CHUNKS)
                    ]
                    for k, t in enumerate(tiles_g):
                        for ci, (c0, cn) in enumerate(CHUNKS):
                            mm = nc.tensor.matmul(
                                pss[ci][:, :],
                                lhs[t][:, 128 * g : 128 * (g + 1)],
                                fts[t][:, c0 : c0 + cn],
                                start=(k == 0),
                                stop=False,
                                skip_group_check=True,
                            )
                            for gx in gates:
                                dep(mm, gx, "matmuls after sample gate")
                            if bgate is not None:
                                dep(mm, bgate, "matmuls after bank gate")
                                dep(mm, bgate2, "matmuls after bank gate2")
                    # pos contribution scaled by len so the final 1/len
                    # multiply leaves exactly pos_table[pos].
                    pc = W * s + 128 * g
                    for ci, (c0, cn) in enumerate(CHUNKS):
                        mm = nc.tensor.matmul(
                            pss[ci][:, :],
                            cpb[0:32, pc : pc + 128],
                            tab[:, c0 : c0 + cn],
                            start=(len(tiles_g) == 0),
                            stop=True,
                            skip_group_check=True,
                        )
                        for gx in gates:
                            dep(mm, gx, "pos matmul after sample gate")
                        if bgate is not None:
                            dep(mm, bgate, "pos matmul after bank gate")
                            dep(mm, bgate2, "pos matmul after bank gate2")
                    for ci, (c0, cn) in enumerate(CHUNKS):
                        unit = 2 * g + ci
                        recip_ap = cpk[
                            :, CPK_WREC + NG * s + g : CPK_WREC + NG * s + g + 1
                        ]
                        if unit % 2 == 0:
                            cp = nc.vector.tensor_scalar(
                                ob[:, g * D + c0 : g * D + c0 + cn],
                                pss[ci][:, :],
                                recip_ap,
                                None,
                                op0=mybir.AluOpType.mult,
                            )
                            if last_dve_copy is not None:
                                dep(cp, last_dve_copy, "DVE copy order")
                            last_dve_copy = cp
                        else:
                            cp = nc.scalar.activation(
                                ob[:, g * D + c0 : g * D + c0 + cn],
                                pss[ci][:, :],
                                mybir.ActivationFunctionType.Copy,
                                scale=recip_ap,
                            )
                            if last_act_copy is not None:
                                dep(cp, last_act_copy, "ACT copy order")
                            last_act_copy = cp
                # Pool probes observe the last DVE copy of each output half
                # on the Pool engine so each output DMA carries only its ACT
                # copy wait.  Two DMAs per sample pipeline the writeback:
                # groups 0-1 ship while groups 2-3 are still being copied.
                nc.gpsimd.tensor_copy(
                    pl_probe[0:1, 2 * s : 2 * s + 1], ob[0:1, D : D + 1]
                )
                nc.gpsimd.dma_start(
                    out=out[s, 0 : 2 * 128].rearrange("(g p) d -> p g d", p=128),
                    in_=ob[:, 0 : 2 * D].rearrange("p (g d) -> p g d", g=2),
                )
                nc.gpsimd.tensor_copy(
                    pl_probe[0:1, 2 * s + 1 : 2 * s + 2],
                    ob[0:1, 3 * D : 3 * D + 1],
                )
                nc.gpsimd.dma_start(
                    out=out[s, 2 * 128 : 4 * 128].rearrange(
                        "(g p) d -> p g d", p=128
                    ),
                    in_=ob[:, 2 * D : 4 * D].rearrange("p (g d) -> p g d", g=2),
                )
                prev_ob = ob
    return nc


_PROGRAM_CACHE = {}


def _get_program(sched):
    key = tuple(tuple(tuple(g) for g in s) for s in sched)
    if key not in _PROGRAM_CACHE:
        _PROGRAM_CACHE[key] = _build_program(sched)
    return _PROGRAM_CACHE[key]


def _assign_slots(spans):
    """Assign the B samples to (slot, core) so that the per-slot UNION of
    (group, char-tile) matmul footprints is small: sort by profile, then
    local-search swaps with incremental per-slot cost."""
    import random

    masks = []
    for i in range(B):
        u = np.zeros((NG, NT), bool)
        for (g, t0, t1) in spans[i][0]:
            u[g, t0 : t1 + 1] = True
        masks.append(u)

    def slot_cost(slot):
        u = np.zeros((NG, NT), bool)
        for i in slot:
            u |= masks[i]
        return int(u.sum())

    order = sorted(range(B), key=lambda i: spans[i][1])
    assign = [[order[s * N_CORES + c] for c in range(N_CORES)] for s in range(SPC)]
    costs = [slot_cost(slot) for slot in assign]
    rng = random.Random(0)
    for _ in range(60000):
        s1, s2 = rng.randrange(SPC), rng.randrange(SPC)
        if s1 == s2:
            continue
        i1, i2 = rng.randrange(N_CORES), rng.randrange(N_CORES)
        assign[s1][i1], assign[s2][i2] = assign[s2][i2], assign[s1][i1]
        c1, c2 = slot_cost(assign[s1]), slot_cost(assign[s2])
        if c1 + c2 <= costs[s1] + costs[s2]:
            costs[s1], costs[s2] = c1, c2
        else:
            assign[s1][i1], assign[s2][i2] = assign[s2][i2], assign[s1][i1]
    return assign


def _prep_inputs(feats, word_lens, seq_len, pos, pos_table):
    """Host-side metadata prep + batch sharding -> per-core input maps,
    union matmul schedule, and the sample->(slot, core) assignment."""
    feats = np.ascontiguousarray(np.asarray(feats), dtype=np.float32)
    word_lens = np.asarray(word_lens).astype(np.int64)
    seq_len = np.asarray(seq_len).astype(np.int64)
    pos = np.asarray(pos).astype(np.int64)
    pos_table = np.ascontiguousarray(np.asarray(pos_table), dtype=np.float32)

    import ml_dtypes

    bf16 = ml_dtypes.bfloat16
    wid = np.full((B, S), -1.0, np.float32)
    wrecw = np.zeros((B, W), np.float32)  # 1/len per word (0 for padding)
    lenw = np.zeros((B, W), np.float32)  # len per word (0 for padding)
    posoh = np.zeros((B, NPOS, W), np.float32)
    spans = []  # per sample: ([(g, t0, t1), ...], profile_key)
    for i in range(B):
        wl = word_lens[i]
        sl = int(seq_len[i])
        valid = wl != 0
        valid[0] = True
        ridx = np.nonzero(valid)[0]  # real words (contiguous prefix by construction)
        starts = wl[ridx]
        n = len(ridx)
        nxt = np.append(starts[1:], 0)
        ends = np.where(nxt == 0, sl, nxt)
        lens = np.maximum(ends - starts, 1)
        cidx = np.arange(sl)
        cwid = np.searchsorted(starts, cidx, side="right") - 1
        wid[i, :sl] = ridx[cwid].astype(np.float32)
        wrecw[i, ridx] = 1.0 / lens.astype(np.float32)
        lenw[i, ridx] = lens.astype(np.float32)
        posoh[i, pos[i], np.arange(W)] = 1.0  # one-hot part
        sp = []
        for g in range(NG):
            w0 = 128 * g
            if w0 >= n:
                continue
            w1 = min(128 * (g + 1), n)
            c0, c1 = int(starts[w0]), int(ends[w1 - 1])
            sp.append((g, c0 // 128, (c1 - 1) // 128))
        spans.append((sp, (n, sl)))

    assign = _assign_slots(spans)
    sched = []
    for s in range(SPC):
        u = np.zeros((NG, NT), bool)
        for i in assign[s]:
            for (g, t0, t1) in spans[i][0]:
                u[g, t0 : t1 + 1] = True
        sched.append(tuple(tuple(np.nonzero(u[g])[0].tolist()) for g in range(NG)))
    sched = tuple(sched)

    # [B, S] -> [B, 128, NT]: per-partition scalar columns per char tile
    widT = wid.reshape(B, NT, 128).transpose(0, 2, 1)
    # 1/len per word -> [B, 128, NG] per-partition scalars per word group
    wrecwT = wrecw.reshape(B, NG, 128).transpose(0, 2, 1)

    feats_bf = feats.astype(bf16)
    tab_bf = np.ascontiguousarray(pos_table.astype(bf16))

    in_maps = []
    for c in range(N_CORES):
        cpk = np.zeros((128, CPK_W), np.float32)
        cpb = np.zeros((32, CPB_W), bf16)
        cpb[:, CPB_TAB : CPB_TAB + D] = tab_bf
        feats_c = np.empty((SPC, S, D), bf16)
        for s in range(SPC):
            i = assign[s][c]
            feats_c[s] = feats_bf[i]
            cpk[:, CPK_META + NT * s : CPK_META + NT * (s + 1)] = widT[i]
            cpk[:, CPK_WREC + NG * s : CPK_WREC + NG * (s + 1)] = wrecwT[i]
            cpb[:, W * s : W * (s + 1)] = (posoh[i] * lenw[i][None, :]).astype(bf16)
        in_maps.append({"feats": feats_c, "constpack": cpk, "constpkb": cpb})
    return in_maps, sched, assign


def _run(in_maps, sched, assign, trace=False):
    from concourse.bass_utils import run_bass_kernel_spmd

    nc = _get_program(sched)
    res = run_bass_kernel_spmd(nc, in_maps, list(range(N_CORES)), trace=trace)
    out = np.empty((B, W, D), np.float32)
    for c in range(N_CORES):
        for s in range(SPC):
            out[assign[s][c]] = res.results[c]["out"][s].astype(np.float32)
    return out, res


def kernel(feats, word_lens, seq_len, pos, pos_table):
    in_maps, sched, assign = _prep_inputs(feats, word_lens, seq_len, pos, pos_table)
    out, _ = _run(in_maps, sched, assign, trace=False)
    return out

